# revision 4
# baseline (speedup 1.0000x reference)
"""Causal GQA attention on 8 TRN2 NeuronCores (Bass/Tile).

Problem: B=2, Tq=Tk=2048, Hq=32, Hkv=8, D=128, fp32, causal softmax(QK^T/sqrt(D))V.

Sharding (tensor parallel over heads): core c handles q-heads [4c, 4c+4) and
kv-head c. Per core that is 8 independent (batch, head) single-head attentions
of Q[2048,128] x K[2048,128] -> V[2048,128]. No cross-core communication.

Per-core kernel layout (host pre-arranges everything into PE-friendly layouts):
  qT   [8, 128, 2048]  f32r  per (b, h_local): Q^T        (d on partitions)
  kT   [2, 128, 2048]  f32r  per batch:        K^T        (d on partitions)
  vp   [2, 128, 16*129] f16  per batch: V tiled [kt, 128, 129] partition-major,
                             col 128 of each tile = 1.0 (fused row-sum column)
  mk   [128, 128]      f16   upper-triangular (c >= p) 0/1 mask for diag tiles
  out  [8, 128, 16*128] f32  per pair: out[qrow, qt*128 + d]

Algorithm per (pair, q-chunk of 512) with k-tile pairs merged into [128,1024]
PSUM tiles (2 banks) so each ACT exp instruction covers 2 k-tiles:
  S^T[kt] = kT[kt-tile].T @ qT-chunk      (fp32r matmul, N=512, full PE rate)
  P^T[kt] = exp(SCALE * S^T[kt])          (ACT, fp16 out; causal: skip columns
                                           left of the diagonal, mask the
                                           diagonal 128x128 block on DVE)
  out[qs] = sum_kt P^T[kt][:, qs-block].T @ Vp[kt]   (fp16 matmul, N=129,
                                           PSUM-accumulated; col 128 = rowsum)
  out_norm = out[:, :128] * reciprocal(out[:, 128])  (DVE)

Chunks are software-pipelined: QK+exp of chunk qc+1 is emitted before the PV
phase of chunk qc so the Scalar engine (the bottleneck) never starves.
"""

import numpy as np

import concourse.bass as bass
import concourse.tile as tile
from concourse import bacc, mybir
from concourse.bass_utils import run_bass_kernel_spmd

B, T, HQ, HKV, D = 2, 2048, 32, 8, 128
SCALE = 0.08838834764831845  # 1/sqrt(128)
NCORES = 8
HL = HQ // NCORES        # q heads per core
PAIRS = B * HL           # (batch, local head) pairs per core
QCH = 512                # q chunk = matmul moving free dim
NQC = T // QCH           # q chunks
KTS = T // 128           # k tiles
f32 = mybir.dt.float32
f32r = mybir.dt.float32r
f16 = mybir.dt.float16

_CACHE = {}


def _build():
    nc = bacc.Bacc("TRN2", target_bir_lowering=False, debug=False, num_devices=NCORES)
    qT = nc.dram_tensor("qT", [PAIRS, 128, T], f32r, kind="ExternalInput").ap()
    kT = nc.dram_tensor("kT", [B, 128, T], f32r, kind="ExternalInput").ap()
    vp = nc.dram_tensor("vp", [B, 128, KTS * 129], f16, kind="ExternalInput").ap()
    mk = nc.dram_tensor("mk", [128, 128], f16, kind="ExternalInput").ap()
    out = nc.dram_tensor("out", [PAIRS, 128, KTS * 128], f32, kind="ExternalOutput").ap()
    EXP = mybir.ActivationFunctionType.Exp

    with tile.TileContext(nc) as tc:
        with tc.tile_pool(name="const", bufs=1) as cst, \
             tc.tile_pool(name="sb", bufs=2) as sbp, \
             tc.tile_pool(name="pt", bufs=16) as ptp, \
             tc.tile_pool(name="st", bufs=3, space="PSUM") as stp, \
             tc.tile_pool(name="po", bufs=2, space="PSUM") as pop:
            # Load order matters for the pipeline ramp: batch-0 data first,
            # and the first two k-tiles of kT[0] ahead of the rest so the
            # first QK matmul can issue ~0.5us in.
            kt0 = cst.tile([128, T], f32r, name="ktc0", tag="ktc0")
            nc.gpsimd.dma_start(out=kt0[:, 0:256], in_=kT[0][:, 0:256])
            mk_sb = cst.tile([128, 128], f16, name="mask", tag="mask")
            nc.gpsimd.dma_start(out=mk_sb[:], in_=mk[:])
            nc.gpsimd.dma_start(out=kt0[:, 256:T], in_=kT[0][:, 256:T])
            vp0 = cst.tile([128, KTS * 129], f16, name="vpc0", tag="vpc0")
            nc.gpsimd.dma_start(out=vp0[:], in_=vp[0])
            kt1 = cst.tile([128, T], f32r, name="ktc1", tag="ktc1")
            nc.gpsimd.dma_start(out=kt1[:], in_=kT[1])
            vp1 = cst.tile([128, KTS * 129], f16, name="vpc1", tag="vpc1")
            nc.gpsimd.dma_start(out=vp1[:], in_=vp[1])
            kts = [kt0, kt1]
            vps = [vp0, vp1]

            def emit_qk_exp(b, q_sb, qc):
                """QK matmuls + exp for one chunk; returns pt tiles (one per
                k-tile *pair*, [128, 1024] f16)."""
                nkt = 4 * qc + 4
                pts = []
                for a in range(nkt // 2):
                    st = stp.tile([128, 1024], f32, name="st", tag="st")
                    for h in range(2):
                        kt = 2 * a + h
                        nc.tensor.matmul(
                            st[:, h * 512:(h + 1) * 512],
                            kts[b][:, kt * 128:(kt + 1) * 128],
                            q_sb[:],
                            start=True, stop=True,
                        )
                    pt = ptp.tile([128, 1024], f16, name="pt", tag="pt")
                    if a < 2 * qc + 1:
                        # fully-valid pair, or the (j0,j1) partial pair: one
                        # exp over the whole [0:1024) range. For (j0,j1) the
                        # j1 columns [512:640) are dead but never read.
                        nc.scalar.activation(pt[:], st[:], EXP, scale=SCALE)
                    else:
                        # (j2,j3) partial pair: two small exps.
                        nc.scalar.activation(pt[:, 256:512], st[:, 256:512], EXP, scale=SCALE)
                        nc.scalar.activation(pt[:, 896:1024], st[:, 896:1024], EXP, scale=SCALE)
                    if a == 2 * qc:      # diag blocks j0, j1
                        nc.vector.tensor_mul(pt[:, 0:128], pt[:, 0:128], mk_sb[:])
                        nc.vector.tensor_mul(pt[:, 640:768], pt[:, 640:768], mk_sb[:])
                    elif a == 2 * qc + 1:  # diag blocks j2, j3
                        nc.vector.tensor_mul(pt[:, 256:384], pt[:, 256:384], mk_sb[:])
                        nc.vector.tensor_mul(pt[:, 896:1024], pt[:, 896:1024], mk_sb[:])
                    pts.append(pt)
                return pts

            def emit_pv(pair, b, qc, pts):
                ost = sbp.tile([128, QCH], f32, name="ost", tag="ost", bufs=3)
                for qs in range(4):
                    gq = 4 * qc + qs
                    ops = pop.tile([128, 129], f32, name="ops", tag="ops")
                    for kt in range(gq + 1):
                        nc.tensor.matmul(
                            ops[:],
                            pts[kt // 2][:, (kt % 2) * 512 + qs * 128:
                                         (kt % 2) * 512 + (qs + 1) * 128],
                            vps[b][:, kt * 129:(kt + 1) * 129],
                            start=(kt == 0), stop=(kt == gq),
                        )
                    rs = sbp.tile([128, 1], f32, name="rs", tag="rs", bufs=6)
                    nc.vector.reciprocal(rs[:], ops[:, 128:129])
                    nc.vector.tensor_scalar_mul(
                        ost[:, qs * 128:(qs + 1) * 128], ops[:, 0:128], rs[:]
                    )
                nc.sync.dma_start(
                    out=out[pair][:, qc * QCH:(qc + 1) * QCH], in_=ost[:]
                )

            prev = None
            for pair in range(PAIRS):
                b = pair // HL
                for qc in range(NQC):
                    q_sb = sbp.tile([128, QCH], f32r, name="q", tag="q", bufs=4)
                    nc.sync.dma_start(
                        out=q_sb[:], in_=qT[pair][:, qc * QCH:(qc + 1) * QCH]
                    )
                    pts = emit_qk_exp(b, q_sb, qc)
                    if prev is not None:
                        emit_pv(prev[0], prev[1], prev[2], prev[3])
                    prev = (pair, b, qc, pts)
            emit_pv(prev[0], prev[1], prev[2], prev[3])
    nc.compile()
    return nc


def _get_nc():
    if "nc" not in _CACHE:
        _CACHE["nc"] = _build()
    return _CACHE["nc"]


def _prep_inputs(q, k, v):
    """Build per-core input maps from full inputs."""
    q = np.asarray(q, dtype=np.float32)
    k = np.asarray(k, dtype=np.float32)
    v = np.asarray(v, dtype=np.float32)
    ones = np.ones((T, 1), dtype=np.float32)
    in_maps = []
    for c in range(NCORES):
        qT = np.empty((PAIRS, 128, T), dtype=np.float32)
        for b in range(B):
            for hl in range(HL):
                qT[b * HL + hl] = q[b, :, HL * c + hl, :].T
        kT = np.empty((B, 128, T), dtype=np.float32)
        vph = np.empty((B, 128, KTS * 129), dtype=np.float16)
        for b in range(B):
            kT[b] = k[b, :, c, :].T
            vcat = np.concatenate([v[b, :, c, :], ones], axis=1)  # [T, 129]
            vph[b] = (
                vcat.reshape(KTS, 128, 129).transpose(1, 0, 2).reshape(128, KTS * 129)
            ).astype(np.float16)
        mkm = (np.arange(128)[None, :] >= np.arange(128)[:, None]).astype(np.float16)
        in_maps.append({
            "qT": np.ascontiguousarray(qT),
            "kT": np.ascontiguousarray(kT),
            "vp": np.ascontiguousarray(vph),
            "mk": mkm,
        })
    return in_maps


def _assemble(results):
    full = np.empty((B, T, HQ * D), dtype=np.float32)
    for c in range(NCORES):
        res = results[c]["out"].reshape(PAIRS, 128, KTS, 128)
        for b in range(B):
            for hl in range(HL):
                h = HL * c + hl
                # [qrow, qt, d] -> [qt*128 + qrow, d]
                full[b, :, h * 128:(h + 1) * 128] = (
                    res[b * HL + hl].transpose(1, 0, 2).reshape(T, 128)
                )
    return full


def kernel(q, k, v):
    nc = _get_nc()
    in_maps = _prep_inputs(q, k, v)
    res = run_bass_kernel_spmd(nc, in_maps, core_ids=list(range(NCORES)))
    return _assemble(res.results)


# revision 6
# speedup vs baseline: 1.0334x; 1.0334x over previous
"""Causal GQA attention on 8 TRN2 NeuronCores (Bass/Tile).

Problem: B=2, Tq=Tk=2048, Hq=32, Hkv=8, D=128, fp32, causal softmax(QK^T/sqrt(D))V.

Sharding (tensor parallel over heads): core c handles q-heads [4c, 4c+4) and
kv-head c. Per core that is 8 independent (batch, head) single-head attentions
of Q[2048,128] x K[2048,128] -> V[2048,128]. No cross-core communication.

Per-core kernel layout (host pre-arranges everything into PE-friendly layouts):
  qT   [8, 128, 2048]  f32r  per (b, h_local): Q^T        (d on partitions)
  kT   [2, 128, 2048]  f32r  per batch:        K^T        (d on partitions)
  vp   [2, 128, 16*129] f16  per batch: V tiled [kt, 128, 129] partition-major,
                             col 128 of each tile = 1.0 (fused row-sum column)
  mk   [128, 128]      f16   upper-triangular (c >= p) 0/1 mask for diag tiles
  out  [8, 128, 16*128] f32  per pair: out[qrow, qt*128 + d]

Algorithm per (pair, q-chunk of 512) with k-tile pairs merged into [128,1024]
PSUM tiles (2 banks) so each ACT exp instruction covers 2 k-tiles:
  S^T[kt] = kT[kt-tile].T @ qT-chunk      (fp32r matmul, N=512, full PE rate)
  P^T[kt] = exp(SCALE * S^T[kt])          (ACT, fp16 out; causal: skip columns
                                           left of the diagonal, mask the
                                           diagonal 128x128 block on DVE)
  out[qs] = sum_kt P^T[kt][:, qs-block].T @ Vp[kt]   (fp16 matmul, N=129,
                                           PSUM-accumulated; col 128 = rowsum)
  out_norm = out[:, :128] * reciprocal(out[:, 128])  (DVE)

Chunks are software-pipelined: QK+exp of chunk qc+1 is emitted before the PV
phase of chunk qc so the Scalar engine (the bottleneck) never starves.
"""

import numpy as np

import concourse.bass as bass
import concourse.tile as tile
from concourse import bacc, mybir
from concourse.bass_utils import run_bass_kernel_spmd

B, T, HQ, HKV, D = 2, 2048, 32, 8, 128
SCALE = 0.08838834764831845  # 1/sqrt(128)
NCORES = 8
HL = HQ // NCORES        # q heads per core
PAIRS = B * HL           # (batch, local head) pairs per core
QCH = 512                # q chunk = matmul moving free dim
NQC = T // QCH           # q chunks
KTS = T // 128           # k tiles
f32 = mybir.dt.float32
f32r = mybir.dt.float32r
f16 = mybir.dt.float16

_CACHE = {}


def _build():
    nc = bacc.Bacc("TRN2", target_bir_lowering=False, debug=False, num_devices=NCORES)
    qT = nc.dram_tensor("qT", [PAIRS, 128, T], f32r, kind="ExternalInput").ap()
    kT = nc.dram_tensor("kT", [B, 128, T], f32r, kind="ExternalInput").ap()
    vp = nc.dram_tensor("vp", [B, 128, KTS * 129], f16, kind="ExternalInput").ap()
    mk = nc.dram_tensor("mk", [128, 128], f16, kind="ExternalInput").ap()
    out = nc.dram_tensor("out", [PAIRS, 128, KTS * 128], f32, kind="ExternalOutput").ap()
    EXP = mybir.ActivationFunctionType.Exp

    with tile.TileContext(nc) as tc:
        with tc.tile_pool(name="const", bufs=1) as cst, \
             tc.tile_pool(name="sb", bufs=2) as sbp, \
             tc.tile_pool(name="pt", bufs=16) as ptp, \
             tc.tile_pool(name="st", bufs=3, space="PSUM") as stp, \
             tc.tile_pool(name="po", bufs=2, space="PSUM") as pop:
            # Load order matters for the pipeline ramp: batch-0 data first,
            # and the first two k-tiles of kT[0] ahead of the rest so the
            # first QK matmul can issue ~0.5us in.
            kt0 = cst.tile([128, T], f32r, name="ktc0", tag="ktc0")
            nc.gpsimd.dma_start(out=kt0[:, 0:256], in_=kT[0][:, 0:256])
            mk_sb = cst.tile([128, 128], f16, name="mask", tag="mask")
            nc.gpsimd.dma_start(out=mk_sb[:], in_=mk[:])
            nc.gpsimd.dma_start(out=kt0[:, 256:T], in_=kT[0][:, 256:T])
            vp0 = cst.tile([128, KTS * 129], f16, name="vpc0", tag="vpc0")
            nc.gpsimd.dma_start(out=vp0[:], in_=vp[0])
            kt1 = cst.tile([128, T], f32r, name="ktc1", tag="ktc1")
            nc.gpsimd.dma_start(out=kt1[:], in_=kT[1])
            vp1 = cst.tile([128, KTS * 129], f16, name="vpc1", tag="vpc1")
            nc.gpsimd.dma_start(out=vp1[:], in_=vp[1])
            kts = [kt0, kt1]
            vps = [vp0, vp1]

            def emit_qk_exp(b, q_sb, qc):
                """QK matmuls + exp for one chunk; returns pt tiles (one per
                k-tile *pair*, [128, 1024] f16)."""
                nkt = 4 * qc + 4
                pts = []
                for a in range(nkt // 2):
                    st = stp.tile([128, 1024], f32, name="st", tag="st")
                    for h in range(2):
                        kt = 2 * a + h
                        nc.tensor.matmul(
                            st[:, h * 512:(h + 1) * 512],
                            kts[b][:, kt * 128:(kt + 1) * 128],
                            q_sb[:],
                            start=True, stop=True,
                        )
                    pt = ptp.tile([128, 1024], f16, name="pt", tag="pt")
                    if a < 2 * qc + 1:
                        # fully-valid pair, or the (j0,j1) partial pair: one
                        # exp over the whole [0:1024) range. For (j0,j1) the
                        # j1 columns [512:640) are dead but never read.
                        nc.scalar.activation(pt[:], st[:], EXP, scale=SCALE)
                    else:
                        # (j2,j3) partial pair: one exp covering both valid
                        # ranges; the dead middle [512:896) is never read.
                        nc.scalar.activation(pt[:, 256:1024], st[:, 256:1024], EXP, scale=SCALE)
                    if a == 2 * qc:      # diag blocks j0, j1
                        nc.vector.tensor_mul(pt[:, 0:128], pt[:, 0:128], mk_sb[:])
                        nc.vector.tensor_mul(pt[:, 640:768], pt[:, 640:768], mk_sb[:])
                    elif a == 2 * qc + 1:  # diag blocks j2, j3
                        nc.vector.tensor_mul(pt[:, 256:384], pt[:, 256:384], mk_sb[:])
                        nc.vector.tensor_mul(pt[:, 896:1024], pt[:, 896:1024], mk_sb[:])
                    pts.append(pt)
                return pts

            def emit_pv(pair, b, qc, pts):
                ost = sbp.tile([128, QCH], f32, name="ost", tag="ost", bufs=3)
                for qs in range(4):
                    gq = 4 * qc + qs
                    ops = pop.tile([128, 129], f32, name="ops", tag="ops")
                    for kt in range(gq + 1):
                        nc.tensor.matmul(
                            ops[:],
                            pts[kt // 2][:, (kt % 2) * 512 + qs * 128:
                                         (kt % 2) * 512 + (qs + 1) * 128],
                            vps[b][:, kt * 129:(kt + 1) * 129],
                            start=(kt == 0), stop=(kt == gq),
                        )
                    rs = sbp.tile([128, 1], f32, name="rs", tag="rs", bufs=6)
                    nc.vector.reciprocal(rs[:], ops[:, 128:129])
                    nc.vector.tensor_scalar_mul(
                        ost[:, qs * 128:(qs + 1) * 128], ops[:, 0:128], rs[:]
                    )
                nc.sync.dma_start(
                    out=out[pair][:, qc * QCH:(qc + 1) * QCH], in_=ost[:]
                )

            prev = None
            for pair in range(PAIRS):
                b = pair // HL
                for qc in range(NQC):
                    q_sb = sbp.tile([128, QCH], f32r, name="q", tag="q", bufs=4)
                    nc.gpsimd.dma_start(
                        out=q_sb[:], in_=qT[pair][:, qc * QCH:(qc + 1) * QCH]
                    )
                    pts = emit_qk_exp(b, q_sb, qc)
                    if prev is not None:
                        emit_pv(prev[0], prev[1], prev[2], prev[3])
                    prev = (pair, b, qc, pts)
            emit_pv(prev[0], prev[1], prev[2], prev[3])
    nc.compile()
    return nc


def _get_nc():
    if "nc" not in _CACHE:
        _CACHE["nc"] = _build()
    return _CACHE["nc"]


def _prep_inputs(q, k, v):
    """Build per-core input maps from full inputs."""
    q = np.asarray(q, dtype=np.float32)
    k = np.asarray(k, dtype=np.float32)
    v = np.asarray(v, dtype=np.float32)
    ones = np.ones((T, 1), dtype=np.float32)
    in_maps = []
    for c in range(NCORES):
        qT = np.empty((PAIRS, 128, T), dtype=np.float32)
        for b in range(B):
            for hl in range(HL):
                qT[b * HL + hl] = q[b, :, HL * c + hl, :].T
        kT = np.empty((B, 128, T), dtype=np.float32)
        vph = np.empty((B, 128, KTS * 129), dtype=np.float16)
        for b in range(B):
            kT[b] = k[b, :, c, :].T
            vcat = np.concatenate([v[b, :, c, :], ones], axis=1)  # [T, 129]
            vph[b] = (
                vcat.reshape(KTS, 128, 129).transpose(1, 0, 2).reshape(128, KTS * 129)
            ).astype(np.float16)
        mkm = (np.arange(128)[None, :] >= np.arange(128)[:, None]).astype(np.float16)
        in_maps.append({
            "qT": np.ascontiguousarray(qT),
            "kT": np.ascontiguousarray(kT),
            "vp": np.ascontiguousarray(vph),
            "mk": mkm,
        })
    return in_maps


def _assemble(results):
    full = np.empty((B, T, HQ * D), dtype=np.float32)
    for c in range(NCORES):
        res = results[c]["out"].reshape(PAIRS, 128, KTS, 128)
        for b in range(B):
            for hl in range(HL):
                h = HL * c + hl
                # [qrow, qt, d] -> [qt*128 + qrow, d]
                full[b, :, h * 128:(h + 1) * 128] = (
                    res[b * HL + hl].transpose(1, 0, 2).reshape(T, 128)
                )
    return full


def kernel(q, k, v):
    nc = _get_nc()
    in_maps = _prep_inputs(q, k, v)
    res = run_bass_kernel_spmd(nc, in_maps, core_ids=list(range(NCORES)))
    return _assemble(res.results)
